# revision 17
# baseline (speedup 1.0000x reference)
"""Trainium2 Bass kernel for nn_BTokenizer (residual MLP tokenizer block).

Computes, for x [16384, 1024]:
    y = x @ Win.T + bin
    6x: y = y + gelu(LN(y) @ Wb[i].T + bb[i])
    out = y @ Wout.T + bout          -> [16384, 2048]

Strategy: data-parallel over tokens across 8 NeuronCores (2048 tokens/core),
weights replicated. Per core, y stays resident in SBUF in [token, feature]
layout; each block does LN (bn_stats + one batched Sqrt to avoid ACT
function-table thrash), PE-transposes the LN output into [feature, token]
tiles, then runs f32r matmuls (full PE rate, ~tf32 precision) accumulating
in PSUM. GELU runs on the scalar engine (kept Gelu-only so its function
table stays warm); residual adds run on GPSIMD to offload the vector engine.

All inputs are pre-transposed/laid out on the host (numpy) inside kernel().
"""

import contextlib

import numpy as np

import concourse.bass as bass
import concourse.tile as tile
from concourse import bacc, mybir
from concourse import bass_utils
from concourse.masks import make_identity

F32 = mybir.dt.float32
F32R = mybir.dt.float32r
BF16 = mybir.dt.bfloat16
AF = mybir.ActivationFunctionType
OP = mybir.AluOpType

N_CORES = 8
N_TOKENS = 16384
T = N_TOKENS // N_CORES  # 2048 tokens per core
D = 1024                 # in/hidden width
NOUT = 2048
NB = 6                   # inner residual blocks
EPS = 1e-5
MT = T // 128            # 16 m-tiles per core
KT = D // 128            # 8 k-tiles


def build_nc(repeat: int = 1, pipeline_stats: bool = False, split_win: bool = False,
             no_transpose_probe: bool = False, dma_t: bool = False, deep_bufs: bool = False):
    nc = bacc.Bacc("TRN2", target_bir_lowering=False, debug=False, num_devices=1)

    xt = nc.dram_tensor("xt", [D, T], F32R, kind="ExternalInput").ap()
    wint = nc.dram_tensor("wint", [D, D], F32R, kind="ExternalInput").ap()
    wbt = nc.dram_tensor("wbt", [NB, D, D], F32R, kind="ExternalInput").ap()
    woutt = nc.dram_tensor("woutt", [D, NOUT], F32R, kind="ExternalInput").ap()
    bin_b = nc.dram_tensor("bin_b", [D], F32, kind="ExternalInput").ap()
    bb = nc.dram_tensor("bb", [NB, D], F32, kind="ExternalInput").ap()
    bout = nc.dram_tensor("bout", [NOUT], F32, kind="ExternalInput").ap()
    out = nc.dram_tensor("out", [T, NOUT], F32, kind="ExternalOutput").ap()

    with tile.TileContext(nc) as tc:
        with contextlib.ExitStack() as ctx:
            kernel_body(ctx, tc, xt, wint, wbt, woutt, bin_b, bb, bout, out, repeat,
                        pipeline_stats, split_win, no_transpose_probe, dma_t,
                        deep_bufs)
    nc.finalize()
    return nc


def _bcast_row(src_ap, parts, free_ap):
    """DRAM AP broadcast across partitions: [[0, parts], *free_ap]."""
    return bass.AP(tensor=src_ap.tensor, offset=src_ap.offset,
                   ap=[[0, parts]] + free_ap)


def kernel_body(ctx, tc, xt, wint, wbt, woutt, bin_b, bb, bout, out, repeat,
                pipeline_stats=True, split_win=True, no_transpose_probe=False,
                dma_t=False, deep_bufs=False):
    nc = tc.nc

    singles = ctx.enter_context(tc.tile_pool(name="singles", bufs=1))
    ypool = ctx.enter_context(tc.tile_pool(name="ypool", bufs=1))
    wpool = ctx.enter_context(tc.tile_pool(name="wpool", bufs=2))
    xpool = ctx.enter_context(tc.tile_pool(name="xpool", bufs=3))
    tpool = ctx.enter_context(tc.tile_pool(name="tpool", bufs=2))
    ttpool = ctx.enter_context(tc.tile_pool(name="ttpool", bufs=3))
    tmppool = ctx.enter_context(tc.tile_pool(name="tmppool", bufs=6 if deep_bufs else 4))
    bbpool = ctx.enter_context(tc.tile_pool(name="bbpool", bufs=2))
    ostpool = ctx.enter_context(tc.tile_pool(name="ostpool", bufs=3))
    statpool = ctx.enter_context(tc.tile_pool(name="statpool", bufs=6))
    psA = ctx.enter_context(tc.tile_pool(name="psA", bufs=4, space="PSUM"))
    psT = ctx.enter_context(tc.tile_pool(name="psT", bufs=4, space="PSUM"))

    # constants
    ident = singles.tile([128, 128], F32)
    make_identity(nc, ident)
    ident_r = singles.tile([128, 128], F32R)
    nc.vector.tensor_copy(ident_r, ident)
    eps = singles.tile([128, 1], F32)
    nc.vector.memset(eps, EPS)
    bin_rep = singles.tile([128, D], F32)
    nc.gpsimd.dma_start(bin_rep, _bcast_row(bin_b, 128, [[1, D]]))
    bout_rep = singles.tile([128, NOUT], F32)
    nc.gpsimd.dma_start(bout_rep, _bcast_row(bout, 128, [[1, NOUT]]))

    # resident y [128, MT, D] fp32
    y_t = ypool.tile([128, MT, D], F32)
    if no_transpose_probe:
        probe_t = singles.tile([128, D], F32R)
        nc.vector.tensor_copy(probe_t, y_t[:, 0, :])

    def emit_stats_m(mvb, m):
        """bn_stats + bn_aggr for one m-tile into mvb[:, m, :]."""
        st = statpool.tile([128, 2, 6], F32)
        nc.vector.bn_stats(st[:, 0, :], y_t[:, m, 0:512])
        nc.vector.bn_stats(st[:, 1, :], y_t[:, m, 512:1024])
        nc.vector.bn_aggr(mvb[:, m, :], st)

    def emit_sqrt_half(mvb, rstdb, half_m):
        sd = statpool.tile([128, 8], F32)
        nc.scalar.activation(sd, mvb[:, bass.ts(half_m, 8), 1], AF.Sqrt, bias=eps)
        nc.vector.reciprocal(rstdb[:, bass.ts(half_m, 8)], sd)

    def new_stats():
        return (statpool.tile([128, MT, 2], F32, name="mvb", tag="mvb"),
                statpool.tile([128, MT], F32, name="rstdb", tag="rstdb"))

    for _rep in range(repeat):
        # ---------------- Phase 1: y = x @ Win.T + bin ----------------
        w_in = wpool.tile([128, KT, D], F32R, tag="w")
        if split_win:  # per-k chunks so the first matmuls start early
            for k in range(KT):
                nc.sync.dma_start(w_in[:, k, :],
                                  wint.rearrange("(kt p) n -> p kt n", p=128)[:, k, :])
        else:
            nc.sync.dma_start(w_in, wint.rearrange("(kt p) n -> p kt n", p=128))
        stats0 = new_stats() if pipeline_stats else None
        for m in range(MT):
            xm = xpool.tile([128, KT, 128], F32R)
            nc.sync.dma_start(xm, xt.rearrange("(kt p) t -> p kt t", p=128)
                              [:, :, bass.ts(m, 128)])
            for n in range(2):
                acc = psA.tile([128, 512], F32)
                for k in range(KT):
                    nc.tensor.matmul(acc, xm[:, k, :], w_in[:, k, bass.ts(n, 512)],
                                     start=(k == 0), stop=(k == KT - 1))
                nc.vector.tensor_tensor(y_t[:, m, bass.ts(n, 512)], acc,
                                        bin_rep[:, bass.ts(n, 512)], OP.add)
            if pipeline_stats:
                emit_stats_m(stats0[0], m)

        # ---------------- Phase 2: residual blocks ----------------
        cur_stats = stats0
        for i in range(NB):
            wb_t = wpool.tile([128, KT, D], F32R, tag="w")
            nc.sync.dma_start(wb_t, wbt[i].rearrange("(kt p) n -> p kt n", p=128))
            bb_rep = bbpool.tile([128, D], F32)
            nc.gpsimd.dma_start(bb_rep, _bcast_row(bb[i], 128, [[1, D]]))

            if cur_stats is None:  # non-pipelined: stats at block start
                cur_stats = new_stats()
                for m in range(MT):
                    emit_stats_m(cur_stats[0], m)
            mvb, rstdb = cur_stats
            # one Sqrt batch per block: 2 ACT table swaps total, emitted at
            # block start so it runs during the previous block's PE tail
            for half_m in range(2):
                emit_sqrt_half(mvb, rstdb, half_m)
            next_stats = (new_stats() if i < NB - 1 else None) if pipeline_stats else None

            for m in range(MT):
                ym = y_t[:, m, :]
                if dma_t:
                    t = tpool.tile([128, D], BF16, name="t", tag="t")
                else:
                    t = tpool.tile([128, D], F32R, name="t", tag="t")
                nc.vector.tensor_scalar(t, ym, mvb[:, m, 0:1], rstdb[:, bass.ds(m, 1)],
                                        OP.subtract, OP.mult)
                tt = ttpool.tile([128, KT, 128], F32R)
                if no_transpose_probe:
                    tt = None  # PERF PROBE ONLY: wrong math, feeds t directly
                elif dma_t:
                    ttb = ttpool.tile([128, KT, 128], BF16, name="ttb", tag="ttb")
                    nc.scalar.dma_start(ttb, t, transpose=True)
                    nc.vector.tensor_copy(tt, ttb)
                else:
                    for kq in range(2):
                        pst = psT.tile([128, 512], F32R, tag="pst")
                        for j in range(4):
                            nc.tensor.transpose(pst[:, bass.ts(j, 128)],
                                                t[:, bass.ts(kq * 4 + j, 128)], ident_r)
                        nc.vector.tensor_copy(tt[:, bass.ts(kq, 4), :].rearrange("p a b -> p (a b)"),
                                              pst)
                for n in range(2):
                    acc = psA.tile([128, 512], F32)
                    for k in range(KT):
                        lhsT = t[:, bass.ts(k, 128)] if tt is None else tt[:, k, :]
                        nc.tensor.matmul(acc, lhsT, wb_t[:, k, bass.ts(n, 512)],
                                         start=(k == 0), stop=(k == KT - 1))
                    tmp = tmppool.tile([128, 512], F32)
                    nc.vector.tensor_tensor(tmp, acc, bb_rep[:, bass.ts(n, 512)], OP.add)
                    nc.scalar.activation(tmp, tmp, AF.Gelu)
                    nc.gpsimd.tensor_tensor(ym[:, bass.ts(n, 512)],
                                            ym[:, bass.ts(n, 512)], tmp, OP.add)
                # pipeline next block's LN stats right behind this m-tile's
                # residual update so they don't queue behind this block's tail
                if next_stats is not None:
                    emit_stats_m(next_stats[0], m)
            cur_stats = next_stats

        # ---------------- Phase 3: out = y @ Wout.T + bout ----------------
        for half in range(2):
            wo = wpool.tile([128, KT, D], F32R, tag="w")
            nc.sync.dma_start(wo, woutt[:, bass.ts(half, D)]
                              .rearrange("(kt p) n -> p kt n", p=128))
            for m in range(MT):
                if no_transpose_probe:
                    tt = None
                elif dma_t:
                    t3 = tpool.tile([128, D], BF16, name="t", tag="t")
                    nc.vector.tensor_copy(t3, y_t[:, m, :])
                    ttb = ttpool.tile([128, KT, 128], BF16, name="ttb", tag="ttb")
                    nc.scalar.dma_start(ttb, t3, transpose=True)
                    tt = ttpool.tile([128, KT, 128], F32R)
                    nc.vector.tensor_copy(tt, ttb)
                else:
                    tt = ttpool.tile([128, KT, 128], F32R)
                    for kq in range(2):
                        pst = psT.tile([128, 512], F32, tag="pst")
                        for j in range(4):
                            nc.tensor.transpose(pst[:, bass.ts(j, 128)],
                                                y_t[:, m, bass.ts(kq * 4 + j, 128)], ident)
                        nc.vector.tensor_copy(tt[:, bass.ts(kq, 4), :].rearrange("p a b -> p (a b)"),
                                              pst)
                ost = ostpool.tile([128, D], F32)
                for n in range(2):
                    acc = psA.tile([128, 512], F32)
                    for k in range(KT):
                        lhsT = (probe_t[:, bass.ts(k, 128)]
                                if tt is None else tt[:, k, :])
                        nc.tensor.matmul(acc, lhsT, wo[:, k, bass.ts(n, 512)],
                                         start=(k == 0), stop=(k == KT - 1))
                    nc.vector.tensor_tensor(ost[:, bass.ts(n, 512)], acc,
                                            bout_rep[:, bass.ds(half * D + n * 512, 512)],
                                            OP.add)
                nc.sync.dma_start(out[bass.ts(m, 128), bass.ts(half, D)], ost)


_CACHED_NC = None


def kernel(x, Win, bin_b, Wb, bb, Wout, bout_b):
    global _CACHED_NC
    x = np.asarray(x, dtype=np.float32)
    if _CACHED_NC is None:
        _CACHED_NC = build_nc()
    nc = _CACHED_NC

    xt_full = np.ascontiguousarray(x.T)                    # [D, N_TOKENS]
    wint = np.ascontiguousarray(np.asarray(Win, np.float32).T)    # [D, D]
    wbt = np.ascontiguousarray(np.asarray(Wb, np.float32).transpose(0, 2, 1))
    woutt = np.ascontiguousarray(np.asarray(Wout, np.float32).T)  # [D, NOUT]
    bin_arr = np.asarray(bin_b, np.float32)
    bb_arr = np.asarray(bb, np.float32)
    bout_arr = np.asarray(bout_b, np.float32)

    in_maps = []
    for c in range(N_CORES):
        in_maps.append({
            "xt": np.ascontiguousarray(xt_full[:, c * T:(c + 1) * T]),
            "wint": wint,
            "wbt": wbt,
            "woutt": woutt,
            "bin_b": bin_arr,
            "bb": bb_arr,
            "bout": bout_arr,
        })
    res = bass_utils.run_bass_kernel_spmd(nc, in_maps, list(range(N_CORES)))
    return np.concatenate([res.results[c]["out"] for c in range(N_CORES)], axis=0)
